# revision 2
# baseline (speedup 1.0000x reference)
"""BitLinear forward (RMSNorm -> int8 quant -> ternary quant -> matmul ->
rescale) on 8 Trainium2 NeuronCores — v10.

Sharding: data-parallel over rows; every core gets the full weight, passed
HOST-TRANSPOSED (wt = weight.T, a pure layout transform like the row
sharding itself). Ternarizing wt tiles directly yields the [k, n] bf16
matmul rhs — no weight transposes on device at all (the DMA-crossbar
transposes cost ~57us of ring time per ring and throttled everything).

w_scale = mean|w| via a streaming |wt| pass (pass 1) then re-stream to
ternarize (pass 2) — no collective (AllReduce round-trip measured
90-155us in every variant).

Queue plan (per-engine streams execute in order at runtime):
 - SYNC:   all DRAM loads, in production order.
 - ACT:    pass-1 |w| sums (Abs+accum), x-chain activations, psum-copy/evac
           halves, all out DMAs.
 - DVE:    x-chain elementwise, inv-scale chain, ternarize u/v/wtn (all
           invb-gated work lives on the engine that produces invb),
           psum-copy/evac halves.
 - PE:     x transposes via identity matmuls (pre-mm, keeps HAM warm), then
           the 1024 main matmuls in a diagonal (q, rt) order matching input
           readiness (8 psum banks rotating).
 - GPSIMD: partition_all_reduce only.

Math (exactness): x_q ints in [-128,127], w_t in {-1,0,1} exact in bf16;
fp32 PSUM accumulation reproduces the fp32 reference einsum bit-for-bit.
RNE via the 1.5*2^23 magic constant; ternary = RNE(clip(w/(s+eps),-1,1)).
"""

import os

import numpy as np

import concourse.bass as bass
import concourse.mybir as mybir
import concourse.tile as tile
from concourse import bacc
from concourse.bass_utils import run_bass_kernel_spmd
from concourse.masks import make_identity
from concourse import bass_isa

F32 = mybir.dt.float32
BF16 = mybir.dt.bfloat16
ALU = mybir.AluOpType
AF = mybir.ActivationFunctionType

N_CORES = 8
R_FULL, K, N = 16384, 1024, 4096
R = R_FULL // N_CORES          # 2048 rows per core
RT = R // 128                  # 16 row tiles per core
KC = K // 128                  # 8 k-chunks
G = N // 512                   # 8 column groups (one psum bank wide)
NH = 16                        # pass-1 half-chunks [128, 2048]

C_MAGIC = 12582912.0
Q_EPS = 1e-5
NORM_EPS = 1e-6


def build_nc(g_is_ones: bool):
    nc = bacc.Bacc("TRN2", target_bir_lowering=False)

    x_d = nc.dram_tensor("x", [R, K], F32, kind="ExternalInput")
    wt_d = nc.dram_tensor("wt", [K, N], F32, kind="ExternalInput")
    if not g_is_ones:
        g_d = nc.dram_tensor("g", [1, K], F32, kind="ExternalInput")
    out_d = nc.dram_tensor("out", [R, N], F32, kind="ExternalOutput")

    with tile.TileContext(nc) as tc:
        with (
            tc.tile_pool(name="persist", bufs=1) as persist,
            tc.tile_pool(name="xp", bufs=5) as x_pool,
            tc.tile_pool(name="uxp", bufs=3) as ux_pool,
            tc.tile_pool(name="xqp", bufs=4) as xq_pool,
            tc.tile_pool(name="w1p", bufs=2) as w1_pool,
            tc.tile_pool(name="w2p", bufs=3) as w2_pool,
            tc.tile_pool(name="uvp", bufs=2) as uv_pool,
            tc.tile_pool(name="osb", bufs=3) as osb_pool,
            tc.tile_pool(name="stats", bufs=2) as st_pool,
            tc.tile_pool(name="pmm", bufs=8, space="PSUM") as psum_mm,
        ):
            # ---- constants / persistent tiles ----
            cb = persist.tile([128, 1], F32, tag="cb")
            nc.vector.memset(cb[:], C_MAGIC)
            ident = persist.tile([128, 128], BF16, tag="ident")
            make_identity(nc, ident[:])

            if not g_is_ones:
                g_row = persist.tile([1, K], F32, tag="g_row")
                nc.sync.dma_start(g_row[:], g_d[:])
                g_b = persist.tile([128, K], F32, tag="g_b")
                nc.gpsimd.partition_broadcast(g_b[:], g_row[0:1, :])

            xqT = [persist.tile([128, KC, 128], BF16, tag=f"xqT{rt}",
                                name=f"xqT{rt}") for rt in range(RT)]
            # ternarized weight quads: (q, t) -> [128, 4, 512] bf16,
            # rhs for matmul (q, j) is wtq[(q, j//4)][:, j%4, :]
            wtq = {(q, t): persist.tile([128, 4, 512], BF16,
                                        tag=f"wtq{q}_{t}",
                                        name=f"wtq{q}_{t}")
                   for q in range(G) for t in range(2)}
            cs_t = [persist.tile([128, 1], F32, tag=f"cs{rt}",
                                 name=f"cs{rt}") for rt in range(RT)]
            xsc_t = [persist.tile([128, 1], F32, tag=f"xsc{rt}",
                                  name=f"xsc{rt}") for rt in range(RT)]
            wpart = persist.tile([128, NH], F32, tag="wpart")
            wsb = persist.tile([128, 1], F32, tag="wsb")
            invb = persist.tile([128, 1], F32, tag="invb")
            xsq_dummy = persist.tile([128, K], BF16, tag="xsq_dummy")
            wabs_dummy = persist.tile([128, 2048], BF16, tag="wabs_dummy")

            # ---- pass 1: streaming |w| sums over [128, 2048] half-chunks ----
            w1_tiles = {}

            def ld_w1(h):
                w1 = w1_pool.tile([128, 2048], F32, tag="w1", name=f"w1_{h}")
                j, half = h // 2, h % 2
                nc.sync.dma_start(
                    w1[:], wt_d[j * 128:(j + 1) * 128,
                                half * 2048:(half + 1) * 2048])
                w1_tiles[h] = w1

            def ar(h):
                with nc.named_scope("w_abs_sum"):
                    if h % 2 == 0:
                        nc.scalar.activation(
                            wabs_dummy[:], w1_tiles[h][:], AF.Abs,
                            accum_out=wpart[:, h:h + 1])
                    else:
                        nc.vector.tensor_reduce(
                            wpart[:, h:h + 1], w1_tiles[h][:],
                            axis=mybir.AxisListType.X,
                            op=ALU.add, apply_absolute_value=True)

            # ---- x pipeline ----
            xt_tiles = {}
            xq_tiles = {}

            def ld_x(rt):
                xt = x_pool.tile([128, K], F32, tag="xt", name=f"xt{rt}")
                nc.sync.dma_start(xt[:], x_d[rt * 128:(rt + 1) * 128, :])
                xt_tiles[rt] = xt

            def chain_x(rt):
                with nc.named_scope("x_quant"):
                    xt = xt_tiles[rt]
                    xg = xt
                    if not g_is_ones:
                        xg = ux_pool.tile([128, K], F32, tag="xg",
                                          name=f"xg{rt}")
                        nc.vector.tensor_mul(xg[:], xt[:], g_b[:])

                    ssq = st_pool.tile([128, 1], F32, tag="ssq")
                    nc.scalar.activation(
                        xsq_dummy[:], xt[:], AF.Square, accum_out=ssq[:])
                    ms = st_pool.tile([128, 1], F32, tag="ms")
                    nc.vector.tensor_scalar(
                        out=ms[:], in0=ssq[:], scalar1=1.0 / K,
                        scalar2=NORM_EPS, op0=ALU.mult, op1=ALU.add)
                    s0 = st_pool.tile([128, 1], F32, tag="s0")
                    nc.scalar.sqrt(s0[:], ms[:])

                    am = st_pool.tile([128, 1], F32, tag="am")
                    nc.vector.tensor_reduce(
                        am[:], xg[:], axis=mybir.AxisListType.X, op=ALU.max,
                        apply_absolute_value=True)
                    r0 = st_pool.tile([128, 1], F32, tag="r0")
                    nc.vector.reciprocal(r0[:], s0[:])
                    t0 = st_pool.tile([128, 1], F32, tag="t0")
                    nc.vector.tensor_mul(t0[:], ms[:], r0[:])
                    t1 = st_pool.tile([128, 1], F32, tag="t1")
                    nc.vector.tensor_add(t1[:], t0[:], s0[:])
                    s1 = st_pool.tile([128, 1], F32, tag="s1")
                    nc.vector.tensor_scalar(
                        out=s1[:], in0=t1[:], scalar1=0.5,
                        scalar2=None, op0=ALU.mult)
                    rs = st_pool.tile([128, 1], F32, tag="rs")
                    nc.vector.reciprocal(rs[:], s1[:])
                    axr = st_pool.tile([128, 1], F32, tag="axr")
                    nc.vector.tensor_mul(axr[:], am[:], rs[:])
                    nc.vector.tensor_scalar(
                        out=xsc_t[rt][:], in0=axr[:], scalar1=1.0 / 127.0,
                        scalar2=None, op0=ALU.mult)
                    sx = st_pool.tile([128, 1], F32, tag="sx")
                    nc.vector.tensor_scalar(
                        out=sx[:], in0=axr[:], scalar1=1.0 / 127.0,
                        scalar2=Q_EPS, op0=ALU.mult, op1=ALU.add)
                    dx = st_pool.tile([128, 1], F32, tag="dx")
                    nc.vector.reciprocal(dx[:], sx[:])
                    srow = st_pool.tile([128, 1], F32, tag="srow")
                    nc.vector.tensor_mul(srow[:], rs[:], dx[:])

                    ux = ux_pool.tile([128, K], F32, tag="ux", name=f"ux{rt}")
                    nc.scalar.activation(
                        ux[:], xg[:], AF.Identity,
                        bias=cb[:, 0:1], scale=srow[:, 0:1])
                    xq = xq_pool.tile([128, K], BF16, tag="xq", name=f"xq{rt}")
                    nc.vector.tensor_scalar(
                        out=xq[:], in0=ux[:], scalar1=C_MAGIC,
                        scalar2=None, op0=ALU.subtract)
                    xq_tiles[rt] = xq

            def emit_cs(rt):
                nc.vector.tensor_mul(cs_t[rt][:], xsc_t[rt][:], wsb[:])

            def emit_T_x(rt):
                # PE identity transpose: xqT[rt][kk, j, r] = xq[r, j*128+kk]
                xq = xq_tiles[rt]
                for h in range(2):
                    tp = psum_mm.tile([128, 512], F32, tag="pmm",
                                      name=f"ptx{rt}_{h}")
                    for c in range(4):
                        j = 4 * h + c
                        nc.tensor.matmul(
                            tp[:, c * 128:(c + 1) * 128],
                            lhsT=xq[:, j * 128:(j + 1) * 128],
                            rhs=ident[:])
                    dst = xqT[rt][:, 4 * h:4 * h + 4, :]
                    if h == 0:
                        nc.vector.tensor_copy(dst, tp[:])
                    else:
                        nc.scalar.copy(dst, tp[:])

            # ---- pass 2: ternarize quads [128, 4, 512] ----
            w2_tiles = {}

            def ld_w2(q, t):
                w2 = w2_pool.tile([128, 4, 512], F32, tag="w2",
                                  name=f"w2_{q}_{t}")
                src = wt_d[4 * t * 128:(4 * t + 4) * 128,
                           q * 512:(q + 1) * 512].rearrange(
                    "(four p) n -> p four n", p=128)
                nc.sync.dma_start(w2[:], src)
                w2_tiles[(q, t)] = w2

            def tern(q, t):
                with nc.named_scope("w_ternarize"):
                    uv = uv_pool.tile([128, 4, 512], F32, tag="uv",
                                      name=f"uv{q}_{t}")
                    nc.vector.tensor_scalar(
                        out=uv[:], in0=w2_tiles[(q, t)][:],
                        scalar1=invb[:, 0:1],
                        scalar2=1.0, op0=ALU.mult, op1=ALU.min)
                    nc.vector.tensor_scalar(
                        out=uv[:], in0=uv[:], scalar1=-1.0,
                        scalar2=C_MAGIC, op0=ALU.max, op1=ALU.add)
                    nc.scalar.activation(
                        wtq[(q, t)][:], uv[:], AF.Copy, bias=-C_MAGIC)

            # ---- matmul + rescale for one (col-group, row-tile) ----
            def emit_mm(q, rt):
                with nc.named_scope("mm"):
                    pst = psum_mm.tile([128, 512], F32, tag="pmm",
                                       name=f"pmm_{q}_{rt}")
                    for j in range(KC):
                        nc.tensor.matmul(
                            pst[:],
                            lhsT=xqT[rt][:, j, :],
                            rhs=wtq[(q, j // 4)][:, j % 4, :],
                            start=(j == 0), stop=(j == KC - 1))
                with nc.named_scope("out_scale"):
                    osbh = osb_pool.tile([128, 512], F32, tag="osb",
                                         name=f"osb{q}_{rt}")
                    if (q + rt) % 2 == 0:
                        nc.scalar.activation(
                            osbh[:], pst[:], AF.Copy,
                            scale=cs_t[rt][:, 0:1])
                    else:
                        nc.vector.tensor_scalar(
                            out=osbh[:], in0=pst[:],
                            scalar1=cs_t[rt][:, 0:1],
                            scalar2=None, op0=ALU.mult)
                    nc.scalar.dma_start(
                        out_d[rt * 128:(rt + 1) * 128,
                              q * 512:(q + 1) * 512],
                        osbh[:])

            # ---- emission schedule ----
            # wave 1: pass-1 loads + |w| sums (ACT), x0-5 with chains (DVE)
            ld_w1(0)
            ld_w1(1)
            ld_x(0)
            ld_x(1)
            ar(0)
            ld_w1(2)
            ld_x(2)
            chain_x(0)
            ar(1)
            ld_w1(3)
            ld_x(3)
            chain_x(1)
            ar(2)
            ld_w1(4)
            ld_x(4)
            chain_x(2)
            ar(3)
            ld_w1(5)
            ld_x(5)
            chain_x(3)
            ar(4)
            ld_w1(6)
            chain_x(4)
            ar(5)
            ld_w1(7)
            chain_x(5)
            for h in range(6, NH):
                ar(h)
                if h + 2 < NH:
                    ld_w1(h + 2)

            # inv-scale chain (DVE)
            wall = st_pool.tile([128, NH], F32, tag="wall")
            nc.gpsimd.partition_all_reduce(
                wall[:], wpart[:], channels=128,
                reduce_op=bass_isa.ReduceOp.add)
            wsumb = st_pool.tile([128, 1], F32, tag="wsumb")
            nc.vector.reduce_sum(wsumb[:], wall[:], axis=mybir.AxisListType.X)
            nc.vector.tensor_scalar(
                out=wsb[:], in0=wsumb[:], scalar1=1.0 / (N * K),
                scalar2=None, op0=ALU.mult)
            speps1 = st_pool.tile([128, 1], F32, tag="speps1")
            nc.vector.tensor_scalar(
                out=speps1[:], in0=wsumb[:], scalar1=1.0 / (N * K),
                scalar2=Q_EPS, op0=ALU.mult, op1=ALU.add)
            nc.vector.reciprocal(invb[:], speps1[:])
            for rt in range(6):
                emit_cs(rt)

            # wave 2: pass-2 quads (group-major) with remaining x merged in
            ld_w2(0, 0)
            ld_w2(0, 1)
            ld_w2(1, 0)
            ld_x(6)
            ld_x(7)
            tern(0, 0)
            chain_x(6)
            emit_cs(6)
            tern(0, 1)
            ld_w2(1, 1)
            ld_x(8)
            chain_x(7)
            emit_cs(7)
            tern(1, 0)
            ld_w2(2, 0)
            ld_x(9)
            chain_x(8)
            emit_cs(8)
            tern(1, 1)
            ld_w2(2, 1)
            ld_x(10)
            chain_x(9)
            emit_cs(9)
            tern(2, 0)
            ld_w2(3, 0)
            ld_x(11)
            chain_x(10)
            emit_cs(10)
            tern(2, 1)
            ld_w2(3, 1)
            ld_x(12)
            chain_x(11)
            emit_cs(11)
            tern(3, 0)
            ld_w2(4, 0)
            ld_x(13)
            chain_x(12)
            emit_cs(12)
            tern(3, 1)
            ld_w2(4, 1)
            ld_x(14)
            chain_x(13)
            emit_cs(13)
            tern(4, 0)
            ld_w2(5, 0)
            ld_x(15)
            chain_x(14)
            emit_cs(14)
            tern(4, 1)
            ld_w2(5, 1)
            chain_x(15)
            emit_cs(15)
            tern(5, 0)
            ld_w2(6, 0)
            tern(5, 1)
            ld_w2(6, 1)
            tern(6, 0)
            ld_w2(7, 0)
            tern(6, 1)
            ld_w2(7, 1)
            tern(7, 0)
            tern(7, 1)

            # ---- mm passes: diagonal (q, rt) order matching readiness ----
            pairs = sorted(
                ((q, rt) for q in range(G) for rt in range(RT)),
                key=lambda p: (max(5.0 * p[0], 1.8 * p[1]), p[0], p[1]))
            transposed = set()
            for q, rt in pairs:
                if rt not in transposed:
                    transposed.add(rt)
                    emit_T_x(rt)
                emit_mm(q, rt)

    nc.compile()
    return nc


def _ensure_ntff_hook():
    """Shim antenv.axon_hooks and install the ctypes NTFF profiling hook."""
    import sys
    import types
    try:
        import antenv.axon_hooks  # noqa: F401
        return
    except ImportError:
        pass
    mod = types.ModuleType("antenv.axon_hooks")
    mod._hook = None
    mod.set_axon_ntff_profile_hook = lambda h: setattr(mod, "_hook", h)
    mod.get_axon_ntff_profile_hook = lambda: mod._hook
    sys.modules["antenv.axon_hooks"] = mod
    import antenv
    antenv.axon_hooks = mod
    try:
        from trn_agent_boot.trn_boot import _ntff_profile_via_ctypes
        hook = _ntff_profile_via_ctypes("/opt/axon/libaxon_pjrt.so")
        if hook is not None:
            mod._hook = hook
    except Exception as e:  # degrade to no-trace
        print(f"ntff hook install failed: {e}")
    import concourse.bass_utils as bu
    bu.upload_artifacts = lambda tmpdir: f"local://{tmpdir}"


_NC_CACHE = {}


def kernel(x: np.ndarray, weight: np.ndarray, norm_weight: np.ndarray) -> np.ndarray:
    x = np.ascontiguousarray(x, dtype=np.float32)
    weight = np.ascontiguousarray(weight, dtype=np.float32)
    norm_weight = np.ascontiguousarray(norm_weight, dtype=np.float32)

    B, S, Kin = x.shape
    xf = x.reshape(-1, Kin)
    wt = np.ascontiguousarray(weight.T)   # layout prep, no compute
    g_is_ones = bool(np.all(norm_weight == 1.0))

    if g_is_ones not in _NC_CACHE:
        _NC_CACHE[g_is_ones] = build_nc(g_is_ones)
    nc = _NC_CACHE[g_is_ones]

    in_maps = []
    for i in range(N_CORES):
        m = {"x": xf[i * R:(i + 1) * R], "wt": wt}
        if not g_is_ones:
            m["g"] = norm_weight.reshape(1, Kin)
        in_maps.append(m)

    trace = bool(int(os.environ.get("BITLIN_TRACE", "0")))
    if trace:
        _ensure_ntff_hook()
    res = run_bass_kernel_spmd(
        nc, in_maps, core_ids=list(range(N_CORES)), trace=trace,
    )
    if trace:
        kernel.last_results = res
    out = np.concatenate([r["out"] for r in res.results], axis=0)
    return out.reshape(B, S, weight.shape[0]).astype(np.float32)


# revision 3
# speedup vs baseline: 1.0620x; 1.0620x over previous
"""BitLinear forward (RMSNorm -> int8 quant -> ternary quant -> matmul ->
rescale) on 8 Trainium2 NeuronCores — v10.

Sharding: data-parallel over rows; every core gets the full weight, passed
HOST-TRANSPOSED (wt = weight.T, a pure layout transform like the row
sharding itself). Ternarizing wt tiles directly yields the [k, n] bf16
matmul rhs — no weight transposes on device at all (the DMA-crossbar
transposes cost ~57us of ring time per ring and throttled everything).

w_scale = mean|w| via a streaming |wt| pass (pass 1) then re-stream to
ternarize (pass 2) — no collective (AllReduce round-trip measured
90-155us in every variant).

Queue plan (per-engine streams execute in order at runtime):
 - SYNC:   all DRAM loads, in production order.
 - ACT:    pass-1 |w| sums (Abs+accum), x-chain activations, psum-copy/evac
           halves, all out DMAs.
 - DVE:    x-chain elementwise, inv-scale chain, ternarize u/v/wtn (all
           invb-gated work lives on the engine that produces invb),
           psum-copy/evac halves.
 - PE:     x transposes via identity matmuls (pre-mm, keeps HAM warm), then
           the 1024 main matmuls in a diagonal (q, rt) order matching input
           readiness (8 psum banks rotating).
 - GPSIMD: partition_all_reduce only.

Math (exactness): x_q ints in [-128,127], w_t in {-1,0,1} exact in bf16;
fp32 PSUM accumulation reproduces the fp32 reference einsum bit-for-bit.
RNE via the 1.5*2^23 magic constant; ternary = RNE(clip(w/(s+eps),-1,1)).
"""

import os

import numpy as np

import concourse.bass as bass
import concourse.mybir as mybir
import concourse.tile as tile
from concourse import bacc
from concourse.bass_utils import run_bass_kernel_spmd
from concourse.masks import make_identity
from concourse import bass_isa

F32 = mybir.dt.float32
BF16 = mybir.dt.bfloat16
ALU = mybir.AluOpType
AF = mybir.ActivationFunctionType

N_CORES = 8
R_FULL, K, N = 16384, 1024, 4096
R = R_FULL // N_CORES          # 2048 rows per core
RT = R // 128                  # 16 row tiles per core
KC = K // 128                  # 8 k-chunks
G = N // 512                   # 8 column groups (one psum bank wide)
NH = 16                        # pass-1 half-chunks [128, 2048]

C_MAGIC = 12582912.0
Q_EPS = 1e-5
NORM_EPS = 1e-6


def build_nc(g_is_ones: bool):
    nc = bacc.Bacc("TRN2", target_bir_lowering=False)

    x_d = nc.dram_tensor("x", [R, K], F32, kind="ExternalInput")
    wt_d = nc.dram_tensor("wt", [K, N], F32, kind="ExternalInput")
    if not g_is_ones:
        g_d = nc.dram_tensor("g", [1, K], F32, kind="ExternalInput")
    out_d = nc.dram_tensor("out", [R, N], F32, kind="ExternalOutput")

    with tile.TileContext(nc) as tc:
        with (
            tc.tile_pool(name="persist", bufs=1) as persist,
            tc.tile_pool(name="xp", bufs=5) as x_pool,
            tc.tile_pool(name="uxp", bufs=3) as ux_pool,
            tc.tile_pool(name="xqp", bufs=4) as xq_pool,
            tc.tile_pool(name="w1p", bufs=2) as w1_pool,
            tc.tile_pool(name="w2p", bufs=3) as w2_pool,
            tc.tile_pool(name="uvp", bufs=2) as uv_pool,
            tc.tile_pool(name="osb", bufs=3) as osb_pool,
            tc.tile_pool(name="stats", bufs=2) as st_pool,
            tc.tile_pool(name="pmm", bufs=8, space="PSUM") as psum_mm,
        ):
            # ---- constants / persistent tiles ----
            cb = persist.tile([128, 1], F32, tag="cb")
            nc.vector.memset(cb[:], C_MAGIC)
            ident = persist.tile([128, 128], BF16, tag="ident")
            make_identity(nc, ident[:])

            if not g_is_ones:
                g_row = persist.tile([1, K], F32, tag="g_row")
                nc.sync.dma_start(g_row[:], g_d[:])
                g_b = persist.tile([128, K], F32, tag="g_b")
                nc.gpsimd.partition_broadcast(g_b[:], g_row[0:1, :])

            xqT = [persist.tile([128, KC, 128], BF16, tag=f"xqT{rt}",
                                name=f"xqT{rt}") for rt in range(RT)]
            # ternarized weight quads: (q, t) -> [128, 4, 512] bf16,
            # rhs for matmul (q, j) is wtq[(q, j//4)][:, j%4, :]
            wtq = {(q, t): persist.tile([128, 4, 512], BF16,
                                        tag=f"wtq{q}_{t}",
                                        name=f"wtq{q}_{t}")
                   for q in range(G) for t in range(2)}
            cs_t = [persist.tile([128, 1], F32, tag=f"cs{rt}",
                                 name=f"cs{rt}") for rt in range(RT)]
            xsc_t = [persist.tile([128, 1], F32, tag=f"xsc{rt}",
                                  name=f"xsc{rt}") for rt in range(RT)]
            wpart = persist.tile([128, NH], F32, tag="wpart")
            wsb = persist.tile([128, 1], F32, tag="wsb")
            invb = persist.tile([128, 1], F32, tag="invb")
            xsq_dummy = persist.tile([128, K], BF16, tag="xsq_dummy")
            wabs_dummy = persist.tile([128, 2048], BF16, tag="wabs_dummy")

            # ---- pass 1: streaming |w| sums over [128, 2048] half-chunks ----
            w1_tiles = {}

            def ld_w1(h):
                w1 = w1_pool.tile([128, 2048], F32, tag="w1", name=f"w1_{h}")
                j, half = h // 2, h % 2
                nc.sync.dma_start(
                    w1[:], wt_d[j * 128:(j + 1) * 128,
                                half * 2048:(half + 1) * 2048])
                w1_tiles[h] = w1

            def ar(h):
                with nc.named_scope("w_abs_sum"):
                    if h % 2 == 0:
                        nc.scalar.activation(
                            wabs_dummy[:], w1_tiles[h][:], AF.Abs,
                            accum_out=wpart[:, h:h + 1])
                    else:
                        nc.vector.tensor_reduce(
                            wpart[:, h:h + 1], w1_tiles[h][:],
                            axis=mybir.AxisListType.X,
                            op=ALU.add, apply_absolute_value=True)

            # ---- x pipeline ----
            xt_tiles = {}
            xq_tiles = {}

            def ld_x(rt):
                xt = x_pool.tile([128, K], F32, tag="xt", name=f"xt{rt}")
                nc.sync.dma_start(xt[:], x_d[rt * 128:(rt + 1) * 128, :])
                xt_tiles[rt] = xt

            def chain_x(rt):
                with nc.named_scope("x_quant"):
                    xt = xt_tiles[rt]
                    xg = xt
                    if not g_is_ones:
                        xg = ux_pool.tile([128, K], F32, tag="xg",
                                          name=f"xg{rt}")
                        nc.vector.tensor_mul(xg[:], xt[:], g_b[:])

                    ssq = st_pool.tile([128, 1], F32, tag="ssq")
                    nc.scalar.activation(
                        xsq_dummy[:], xt[:], AF.Square, accum_out=ssq[:])
                    ms = st_pool.tile([128, 1], F32, tag="ms")
                    nc.vector.tensor_scalar(
                        out=ms[:], in0=ssq[:], scalar1=1.0 / K,
                        scalar2=NORM_EPS, op0=ALU.mult, op1=ALU.add)
                    s0 = st_pool.tile([128, 1], F32, tag="s0")
                    nc.scalar.sqrt(s0[:], ms[:])

                    am = st_pool.tile([128, 1], F32, tag="am")
                    nc.vector.tensor_reduce(
                        am[:], xg[:], axis=mybir.AxisListType.X, op=ALU.max,
                        apply_absolute_value=True)
                    r0 = st_pool.tile([128, 1], F32, tag="r0")
                    nc.vector.reciprocal(r0[:], s0[:])
                    t0 = st_pool.tile([128, 1], F32, tag="t0")
                    nc.vector.tensor_mul(t0[:], ms[:], r0[:])
                    t1 = st_pool.tile([128, 1], F32, tag="t1")
                    nc.vector.tensor_add(t1[:], t0[:], s0[:])
                    s1 = st_pool.tile([128, 1], F32, tag="s1")
                    nc.vector.tensor_scalar(
                        out=s1[:], in0=t1[:], scalar1=0.5,
                        scalar2=None, op0=ALU.mult)
                    rs = st_pool.tile([128, 1], F32, tag="rs")
                    nc.vector.reciprocal(rs[:], s1[:])
                    axr = st_pool.tile([128, 1], F32, tag="axr")
                    nc.vector.tensor_mul(axr[:], am[:], rs[:])
                    nc.vector.tensor_scalar(
                        out=xsc_t[rt][:], in0=axr[:], scalar1=1.0 / 127.0,
                        scalar2=None, op0=ALU.mult)
                    sx = st_pool.tile([128, 1], F32, tag="sx")
                    nc.vector.tensor_scalar(
                        out=sx[:], in0=axr[:], scalar1=1.0 / 127.0,
                        scalar2=Q_EPS, op0=ALU.mult, op1=ALU.add)
                    dx = st_pool.tile([128, 1], F32, tag="dx")
                    nc.vector.reciprocal(dx[:], sx[:])
                    srow = st_pool.tile([128, 1], F32, tag="srow")
                    nc.vector.tensor_mul(srow[:], rs[:], dx[:])

                    ux = ux_pool.tile([128, K], F32, tag="ux", name=f"ux{rt}")
                    nc.scalar.activation(
                        ux[:], xg[:], AF.Identity,
                        bias=cb[:, 0:1], scale=srow[:, 0:1])
                    xq = xq_pool.tile([128, K], BF16, tag="xq", name=f"xq{rt}")
                    nc.vector.tensor_scalar(
                        out=xq[:], in0=ux[:], scalar1=C_MAGIC,
                        scalar2=None, op0=ALU.subtract)
                    xq_tiles[rt] = xq

            def emit_cs(rt):
                nc.vector.tensor_mul(cs_t[rt][:], xsc_t[rt][:], wsb[:])

            def emit_T_x(rt):
                # PE identity transpose: xqT[rt][kk, j, r] = xq[r, j*128+kk]
                xq = xq_tiles[rt]
                for h in range(2):
                    tp = psum_mm.tile([128, 512], F32, tag="pmm",
                                      name=f"ptx{rt}_{h}")
                    for c in range(4):
                        j = 4 * h + c
                        nc.tensor.matmul(
                            tp[:, c * 128:(c + 1) * 128],
                            lhsT=xq[:, j * 128:(j + 1) * 128],
                            rhs=ident[:])
                    dst = xqT[rt][:, 4 * h:4 * h + 4, :]
                    if h == 0:
                        nc.vector.tensor_copy(dst, tp[:])
                    else:
                        nc.scalar.copy(dst, tp[:])

            # ---- pass 2: ternarize quads [128, 4, 512] ----
            w2_tiles = {}

            def ld_w2(q, t):
                w2 = w2_pool.tile([128, 4, 512], F32, tag="w2",
                                  name=f"w2_{q}_{t}")
                src = wt_d[4 * t * 128:(4 * t + 4) * 128,
                           q * 512:(q + 1) * 512].rearrange(
                    "(four p) n -> p four n", p=128)
                nc.sync.dma_start(w2[:], src)
                w2_tiles[(q, t)] = w2

            def tern(q, t):
                with nc.named_scope("w_ternarize"):
                    uv = uv_pool.tile([128, 4, 512], F32, tag="uv",
                                      name=f"uv{q}_{t}")
                    nc.vector.tensor_scalar(
                        out=uv[:], in0=w2_tiles[(q, t)][:],
                        scalar1=invb[:, 0:1],
                        scalar2=1.0, op0=ALU.mult, op1=ALU.min)
                    nc.vector.tensor_scalar(
                        out=uv[:], in0=uv[:], scalar1=-1.0,
                        scalar2=C_MAGIC, op0=ALU.max, op1=ALU.add)
                    nc.scalar.activation(
                        wtq[(q, t)][:], uv[:], AF.Copy, bias=-C_MAGIC)

            # ---- matmul + rescale for one (col-group, row-tile) ----
            def emit_mm(q, rt):
                with nc.named_scope("mm"):
                    pst = psum_mm.tile([128, 512], F32, tag="pmm",
                                       name=f"pmm_{q}_{rt}")
                    for j in range(KC):
                        nc.tensor.matmul(
                            pst[:],
                            lhsT=xqT[rt][:, j, :],
                            rhs=wtq[(q, j // 4)][:, j % 4, :],
                            start=(j == 0), stop=(j == KC - 1))
                with nc.named_scope("out_scale"):
                    osbh = osb_pool.tile([128, 512], F32, tag="osb",
                                         name=f"osb{q}_{rt}")
                    if (q + rt) % 2 == 0:
                        nc.scalar.activation(
                            osbh[:], pst[:], AF.Copy,
                            scale=cs_t[rt][:, 0:1])
                    else:
                        nc.vector.tensor_scalar(
                            out=osbh[:], in0=pst[:],
                            scalar1=cs_t[rt][:, 0:1],
                            scalar2=None, op0=ALU.mult)
                    nc.scalar.dma_start(
                        out_d[rt * 128:(rt + 1) * 128,
                              q * 512:(q + 1) * 512],
                        osbh[:])

            # ---- emission schedule ----
            # wave 1: pure pass-1 (loads ring-paced, sums alternate ACT/DVE)
            ld_w1(0)
            ld_w1(1)
            for h in range(NH):
                ar(h)
                if h + 2 < NH:
                    ld_w1(h + 2)

            # inv-scale chain (DVE)
            wall = st_pool.tile([128, NH], F32, tag="wall")
            nc.gpsimd.partition_all_reduce(
                wall[:], wpart[:], channels=128,
                reduce_op=bass_isa.ReduceOp.add)
            wsumb = st_pool.tile([128, 1], F32, tag="wsumb")
            nc.vector.reduce_sum(wsumb[:], wall[:], axis=mybir.AxisListType.X)
            nc.vector.tensor_scalar(
                out=wsb[:], in0=wsumb[:], scalar1=1.0 / (N * K),
                scalar2=None, op0=ALU.mult)
            speps1 = st_pool.tile([128, 1], F32, tag="speps1")
            nc.vector.tensor_scalar(
                out=speps1[:], in0=wsumb[:], scalar1=1.0 / (N * K),
                scalar2=Q_EPS, op0=ALU.mult, op1=ALU.add)
            nc.vector.reciprocal(invb[:], speps1[:])

            # wave 2: terns + x chains merged, with mm pairs (+ evac/out)
            # spliced in as soon as their inputs have been emitted
            pairs = sorted(
                ((q, rt) for q in range(G) for rt in range(RT)),
                key=lambda p: (max(5.0 * p[0], 1.8 * p[1]), p[0], p[1]))
            pending = list(pairs)
            transposed = set()
            groups_done = 0
            chains_done = -1

            def drain_pairs(budget):
                nonlocal_state = 0
                emitted = 0
                i = 0
                while emitted < budget and i < len(pending):
                    q, rt = pending[i]
                    if q < groups_done and rt <= chains_done:
                        if rt not in transposed:
                            transposed.add(rt)
                            emit_T_x(rt)
                        emit_mm(q, rt)
                        pending.pop(i)
                        emitted += 1
                    else:
                        i += 1

            ld_x(0)
            ld_x(1)
            ld_x(2)
            ld_w2(0, 0)
            ld_w2(0, 1)
            ld_w2(1, 0)
            tern_seq = [(q, t) for q in range(G) for t in range(2)]
            for k in range(16):
                q, t = tern_seq[k]
                tern(q, t)
                if t == 1:
                    groups_done = q + 1
                if k + 3 < 16:
                    ld_w2(*tern_seq[k + 3])
                chain_x(k)
                emit_cs(k)
                chains_done = k
                if k + 3 < RT:
                    ld_x(k + 3)
                drain_pairs(3)
            for q, rt in pending:
                if rt not in transposed:
                    transposed.add(rt)
                    emit_T_x(rt)
                emit_mm(q, rt)

    nc.compile()
    return nc


def _ensure_ntff_hook():
    """Shim antenv.axon_hooks and install the ctypes NTFF profiling hook."""
    import sys
    import types
    try:
        import antenv.axon_hooks  # noqa: F401
        return
    except ImportError:
        pass
    mod = types.ModuleType("antenv.axon_hooks")
    mod._hook = None
    mod.set_axon_ntff_profile_hook = lambda h: setattr(mod, "_hook", h)
    mod.get_axon_ntff_profile_hook = lambda: mod._hook
    sys.modules["antenv.axon_hooks"] = mod
    import antenv
    antenv.axon_hooks = mod
    try:
        from trn_agent_boot.trn_boot import _ntff_profile_via_ctypes
        hook = _ntff_profile_via_ctypes("/opt/axon/libaxon_pjrt.so")
        if hook is not None:
            mod._hook = hook
    except Exception as e:  # degrade to no-trace
        print(f"ntff hook install failed: {e}")
    import concourse.bass_utils as bu
    bu.upload_artifacts = lambda tmpdir: f"local://{tmpdir}"


_NC_CACHE = {}


def kernel(x: np.ndarray, weight: np.ndarray, norm_weight: np.ndarray) -> np.ndarray:
    x = np.ascontiguousarray(x, dtype=np.float32)
    weight = np.ascontiguousarray(weight, dtype=np.float32)
    norm_weight = np.ascontiguousarray(norm_weight, dtype=np.float32)

    B, S, Kin = x.shape
    xf = x.reshape(-1, Kin)
    wt = np.ascontiguousarray(weight.T)   # layout prep, no compute
    g_is_ones = bool(np.all(norm_weight == 1.0))

    if g_is_ones not in _NC_CACHE:
        _NC_CACHE[g_is_ones] = build_nc(g_is_ones)
    nc = _NC_CACHE[g_is_ones]

    in_maps = []
    for i in range(N_CORES):
        m = {"x": xf[i * R:(i + 1) * R], "wt": wt}
        if not g_is_ones:
            m["g"] = norm_weight.reshape(1, Kin)
        in_maps.append(m)

    trace = bool(int(os.environ.get("BITLIN_TRACE", "0")))
    if trace:
        _ensure_ntff_hook()
    res = run_bass_kernel_spmd(
        nc, in_maps, core_ids=list(range(N_CORES)), trace=trace,
    )
    if trace:
        kernel.last_results = res
    out = np.concatenate([r["out"] for r in res.results], axis=0)
    return out.reshape(B, S, weight.shape[0]).astype(np.float32)
